# revision 17
# baseline (speedup 1.0000x reference)
"""Cross-entropy loss (nn_CrossEntropyLoss) on 8 Trainium2 NeuronCores.

Reference computation (full shapes):
    predicts: [4096, 32000] f32, targets: [4096] int64
    loss = mean_i( log(sum_j exp(predicts[i, j])) - predicts[i, targets[i]] )

Strategy: data-parallel over the batch dim; fp8 input stream; the
sum-of-exp work is column-split between the ACT and DVE engines.

The host rounds predicts to fp8 e4m3 before upload (quartering HBM
traffic to 16 MB/core; the serial input DMA takes ~40us and never
binds).  The exp+row-sum is the real cost: ACT computes exp at a
dtype-independent 1 elem/cycle/lane (153.6 G elem/s), so each core's
16.4M elements would take ~107us on ACT alone.  Each [128, w] chunk is
therefore column-split:
  - ACT: exact exp with accum_out on the left w-wV columns (output to
    a throwaway bf16 scratch; the f32 accumulator is what we keep)
  - DVE: Schraudolph bit-trick exp on the right wV columns --
    tensor_scalar (x*EXP_A + EXP_B) written as int16 produces the
    bf16 bit pattern of ~exp(x) (2x_2p mode, 0.53 cyc/elem); the
    bitcast-bf16 view is then pairwise-added twice (tensor_tensor at
    bf16 2x_1p: 0.28 + 0.14 cyc/elem) and the remaining quarter summed
    by the 1x accumulating tensor_scalar (0.27 cyc/elem) -- ~1.22
    cyc/elem total, ~100 G elem/s of extra throughput on an otherwise
    idle engine.
The split ratio balances the measured engine rates (ACT 0.833 ns/col +
0.38us/chunk vs DVE 1.17 ns/col + 0.5us/chunk -> ~42% to DVE).  Block 0
ramps chunk widths, with its DMAs issued up-front alternating across
the two HWDGE rings (sync + scalar queues) so the early chunks land
pairwise and both engines start ~10.5us in.  Per-chunk partial row-sums
land in one [128, 2*NCHUNK] f32 tile; all but the last chunk's columns
are DMA'd out under the final chunk's compute, and the last two columns
go out on the scalar queue right after the last accumulator read.

Host side (f64): log of the row-sums (logsumexp), the gather of
predicts[i, targets[i]] from the exact f32 input, and the final mean --
the scalar "all-reduce" across the 8 cores.

Accuracy: fp8 quantizes each logit to ~2^-3.5 relative; the softmax-
weighted average of those i.i.d. perturbations shifts each row's lse by
only ~4e-4 (the 12k-effective-term sum averages them out).  EXP_B is
calibrated so the softmax-weighted mean of approx/true - 1 is zero; the
residual +-3% Schraudolph wiggle averages out the same way.  Measured
end-to-end loss relative error ~1e-4 (tolerance 2e-2).
"""

import sys

import numpy as np

sys.path.insert(0, "/opt/trn_rl_repo")

BATCH = 4096
C = 32000
NCORES = 8
R = BATCH // NCORES  # 512 rows per core
P = 128
NBLK = R // P  # 4 row blocks per core
CH = 16000  # max ramp chunk (16 KiB/partition in fp8)
CHB = 32000  # steady chunk: one whole row-block per chunk

# (width, dve_width) per chunk, per block
_S = (CHB, 13232)
BLOCK_SPECS = [
    [(3000, 1264), (3000, 1264), (6000, 2528), (6000, 2528), (14000, 5888)],
    [_S],
    [_S],
    [_S],
]
assert all(sum(w for w, _ in blk) == C for blk in BLOCK_SPECS)
assert all(wv % 16 == 0 for blk in BLOCK_SPECS for _, wv in blk)
NCHUNK = sum(len(blk) for blk in BLOCK_SPECS)
WVMAX = max(wv for blk in BLOCK_SPECS for _, wv in blk)
WAMAX = max(w - wv for blk in BLOCK_SPECS for w, wv in blk)

# Schraudolph exp in bf16: exp(x) ~= bitcast_bf16(int16(x * EXP_A + EXP_B)).
# EXP_A = 2^7/ln2 (bf16 exponent layout = f32's, 7-bit mantissa); EXP_B
# calibrated (numpy, softmax-weighted) so the approximation is
# mean-unbiased inside a row sum of exps.
EXP_A = 184.6649652337873
EXP_B = 16249.0

_CACHE: dict = {}


def _build_nc():
    import concourse.bacc as bacc
    import concourse.tile as tile
    from concourse import mybir

    nc = bacc.Bacc(
        "TRN2", target_bir_lowering=False, debug=False, num_devices=NCORES
    )
    x = nc.dram_tensor("x", [R, C], mybir.dt.float8e4, kind="ExternalInput")
    s = nc.dram_tensor(
        "s", [P, 2 * NCHUNK], mybir.dt.float32, kind="ExternalOutput"
    )

    with tile.TileContext(nc) as tc:
        with (
            tc.tile_pool(name="xch", bufs=4) as xpool,
            tc.tile_pool(name="xbig", bufs=2) as bpool,
            tc.tile_pool(name="eout", bufs=1) as epool,
            tc.tile_pool(name="bits", bufs=1) as vpool,
            tc.tile_pool(name="half", bufs=1) as hpool,
            tc.tile_pool(name="small", bufs=1) as spool,
        ):
            sums = spool.tile([P, 2 * NCHUNK], mybir.dt.float32, tag="sums")

            def compute(xt, w, wv, idx):
                    wa = w - wv
                    # ACT: exact exp on the left wa columns (the fp8
                    # throwaway output may saturate/round; the accumulator
                    # we keep is computed in f32 upstream of the write)
                    et = epool.tile([P, WAMAX], mybir.dt.float8e4, tag="et")
                    nc.scalar.activation(
                        out=et[:, :wa],
                        in_=xt[:, :wa],
                        func=mybir.ActivationFunctionType.Exp,
                        accum_out=sums[:, 2 * idx : 2 * idx + 1],
                    )
                    # DVE: Schraudolph exp bits for the right wv columns
                    sc = vpool.tile([P, WVMAX], mybir.dt.int16, tag="sc")
                    nc.vector.tensor_scalar(
                        out=sc[:, :wv],
                        in0=xt[:, wa:w],
                        scalar1=EXP_A,
                        scalar2=EXP_B,
                        op0=mybir.AluOpType.mult,
                        op1=mybir.AluOpType.add,
                    )
                    # two pairwise-add halvings at bf16 2x, then the 1x
                    # accumulating reduce touches only wv/4 elements
                    scf = sc[:, :wv].bitcast(mybir.dt.bfloat16)
                    ht = hpool.tile([P, WVMAX // 2], mybir.dt.bfloat16, tag="ht")
                    nc.vector.tensor_tensor(
                        out=ht[:, : wv // 2],
                        in0=scf[:, : wv // 2],
                        in1=scf[:, wv // 2 :],
                        op=mybir.AluOpType.add,
                    )
                    nc.vector.tensor_tensor(
                        out=ht[:, : wv // 4],
                        in0=ht[:, : wv // 4],
                        in1=ht[:, wv // 4 : wv // 2],
                        op=mybir.AluOpType.add,
                    )
                    nc.vector.tensor_tensor(
                        out=ht[:, : wv // 8],
                        in0=ht[:, : wv // 8],
                        in1=ht[:, wv // 8 : wv // 4],
                        op=mybir.AluOpType.add,
                    )
                    nc.vector.tensor_scalar(
                        out=ht[:, : wv // 8],
                        in0=ht[:, : wv // 8],
                        scalar1=1.0,
                        scalar2=None,
                        op0=mybir.AluOpType.mult,
                        op1=mybir.AluOpType.add,  # accum reduce op
                        accum_out=sums[:, 2 * idx + 1 : 2 * idx + 2],
                    )
                    if idx == NCHUNK - 2:
                        # everything but the last chunk's two accum columns:
                        # overlaps the final chunk's compute
                        nc.sync.dma_start(
                            out=s[:, : 2 * (NCHUNK - 1)],
                            in_=sums[:, : 2 * (NCHUNK - 1)],
                        )

            idx = 0
            for b in range(NBLK):
                col = 0
                if b == 0:
                    # ramp: issue all block-0 DMAs up front, alternating the
                    # two HWDGE rings (sync + scalar queues) so early chunks
                    # land pairwise instead of serializing on one ring
                    tiles = []
                    for j, (w, wv) in enumerate(BLOCK_SPECS[0]):
                        xt = xpool.tile([P, CH], mybir.dt.float8e4, tag="xt")
                        eng = nc.scalar if j % 2 == 1 else nc.sync
                        eng.dma_start(out=xt[:, :w], in_=x[:P, col : col + w])
                        tiles.append((xt, w, wv))
                        col += w
                    for xt, w, wv in tiles:
                        compute(xt, w, wv, idx)
                        idx += 1
                else:
                    for w, wv in BLOCK_SPECS[b]:
                        xt = bpool.tile([P, CHB], mybir.dt.float8e4, tag="xb")
                        nc.sync.dma_start(
                            out=xt[:, :w],
                            in_=x[b * P : (b + 1) * P, col : col + w],
                        )
                        compute(xt, w, wv, idx)
                        col += w
                        idx += 1
            nc.scalar.dma_start(
                out=s[:, 2 * (NCHUNK - 1) :], in_=sums[:, 2 * (NCHUNK - 1) :]
            )
    nc.compile()
    return nc


def get_nc():
    if "nc" not in _CACHE:
        _CACHE["nc"] = _build_nc()
    return _CACHE["nc"]


def make_in_maps(predicts: np.ndarray, targets: np.ndarray) -> list[dict]:
    import ml_dtypes

    predicts = np.ascontiguousarray(predicts, dtype=np.float32)
    xq = predicts.astype(ml_dtypes.float8_e4m3)  # RTNE
    return [
        {"x": np.ascontiguousarray(xq[c * R : (c + 1) * R])} for c in range(NCORES)
    ]


def kernel(predicts: np.ndarray, targets: np.ndarray) -> np.ndarray:
    from concourse.bass_utils import run_bass_kernel_spmd

    nc = get_nc()
    predicts = np.ascontiguousarray(predicts, dtype=np.float32)
    targets = np.asarray(targets).astype(np.int64)
    in_maps = make_in_maps(predicts, targets)
    res = run_bass_kernel_spmd(nc, in_maps, list(range(NCORES)))

    # chunk -> block column groups in the [P, 2*NCHUNK] sums output
    bounds = np.cumsum([0] + [len(blk) for blk in BLOCK_SPECS])
    total = np.float64(0.0)
    for c in range(NCORES):
        s = np.asarray(res.results[c]["s"], dtype=np.float64)  # [P, 2*NCHUNK]
        for b in range(NBLK):
            rowsum = s[:, 2 * bounds[b] : 2 * bounds[b + 1]].sum(axis=1)  # [P]
            total += np.log(rowsum).sum()
    picked = predicts[np.arange(BATCH), targets].astype(np.float64)
    return np.asarray((total - picked.sum()) / BATCH, dtype=np.float32)


# revision 18
# speedup vs baseline: 1.1309x; 1.1309x over previous
"""Cross-entropy loss (nn_CrossEntropyLoss) on 8 Trainium2 NeuronCores.

Reference computation (full shapes):
    predicts: [4096, 32000] f32, targets: [4096] int64
    loss = mean_i( log(sum_j exp(predicts[i, j])) - predicts[i, targets[i]] )

Strategy: data-parallel over the batch dim; fp8 input stream; the
sum-of-exp work is column-split between the ACT and DVE engines.

The host rounds predicts to fp8 e4m3 before upload (quartering HBM
traffic to 16 MB/core; the serial input DMA takes ~40us and never
binds).  The exp+row-sum is the real cost: ACT computes exp at a
dtype-independent 1 elem/cycle/lane (153.6 G elem/s), so each core's
16.4M elements would take ~107us on ACT alone.  Each [128, w] chunk is
therefore column-split:
  - ACT: exact exp with accum_out on the left w-wV columns (output to
    a throwaway bf16 scratch; the f32 accumulator is what we keep)
  - DVE: Schraudolph bit-trick exp on the right wV columns --
    tensor_scalar (x*EXP_A + EXP_B) written as int16 produces the
    bf16 bit pattern of ~exp(x) (2x_2p mode, 0.53 cyc/elem); the
    bitcast-bf16 view is then pairwise-added twice (tensor_tensor at
    bf16 2x_1p: 0.28 + 0.14 cyc/elem) and the remaining quarter summed
    by the 1x accumulating tensor_scalar (0.27 cyc/elem) -- ~1.22
    cyc/elem total, ~100 G elem/s of extra throughput on an otherwise
    idle engine.
The split ratio balances the measured engine rates (ACT 0.833 ns/col +
0.38us/chunk vs DVE 1.17 ns/col + 0.5us/chunk -> ~42% to DVE).  Block 0
ramps chunk widths, with its DMAs issued up-front alternating across
the two HWDGE rings (sync + scalar queues) so the early chunks land
pairwise and both engines start ~10.5us in.  Per-chunk partial row-sums
land in one [128, 2*NCHUNK] f32 tile; all but the last chunk's columns
are DMA'd out under the final chunk's compute, and the last two columns
go out on the scalar queue right after the last accumulator read.

Host side (f64): log of the row-sums (logsumexp), the gather of
predicts[i, targets[i]] from the exact f32 input, and the final mean --
the scalar "all-reduce" across the 8 cores.

Accuracy: fp8 quantizes each logit to ~2^-3.5 relative; the softmax-
weighted average of those i.i.d. perturbations shifts each row's lse by
only ~4e-4 (the 12k-effective-term sum averages them out).  EXP_B is
calibrated so the softmax-weighted mean of approx/true - 1 is zero; the
residual +-3% Schraudolph wiggle averages out the same way.  Measured
end-to-end loss relative error ~1e-4 (tolerance 2e-2).
"""

import sys

import numpy as np

sys.path.insert(0, "/opt/trn_rl_repo")

BATCH = 4096
C = 32000
NCORES = 8
R = BATCH // NCORES  # 512 rows per core
P = 128
NBLK = R // P  # 4 row blocks per core
CH = 16000  # max column chunk (16 KiB/partition in fp8)

# (width, dve_width) per chunk, per block
_S = (CH, 6736)
BLOCK_SPECS = [
    [(3000, 1264), (3000, 1264), (6000, 2528), (6000, 2528), (14000, 5888)],
    [_S, _S],
    [_S, _S],
    [_S, _S],
]
assert all(sum(w for w, _ in blk) == C for blk in BLOCK_SPECS)
assert all(wv % 16 == 0 for blk in BLOCK_SPECS for _, wv in blk)
NCHUNK = sum(len(blk) for blk in BLOCK_SPECS)
WVMAX = max(wv for blk in BLOCK_SPECS for _, wv in blk)
WAMAX = max(w - wv for blk in BLOCK_SPECS for w, wv in blk)

# Schraudolph exp in bf16: exp(x) ~= bitcast_bf16(int16(x * EXP_A + EXP_B)).
# EXP_A = 2^7/ln2 (bf16 exponent layout = f32's, 7-bit mantissa); EXP_B
# calibrated (numpy, softmax-weighted) so the approximation is
# mean-unbiased inside a row sum of exps.
EXP_A = 184.6649652337873
EXP_B = 16249.0

_CACHE: dict = {}


def _build_nc():
    import concourse.bacc as bacc
    import concourse.tile as tile
    from concourse import mybir

    nc = bacc.Bacc(
        "TRN2", target_bir_lowering=False, debug=False, num_devices=NCORES
    )
    x = nc.dram_tensor("x", [R, C], mybir.dt.float8e4, kind="ExternalInput")
    s = nc.dram_tensor(
        "s", [P, 2 * NCHUNK], mybir.dt.float32, kind="ExternalOutput"
    )

    with tile.TileContext(nc) as tc:
        with (
            tc.tile_pool(name="xch", bufs=6) as xpool,
            tc.tile_pool(name="eout", bufs=2) as epool,
            tc.tile_pool(name="bits", bufs=2) as vpool,
            tc.tile_pool(name="half", bufs=2) as hpool,
            tc.tile_pool(name="small", bufs=1) as spool,
        ):
            sums = spool.tile([P, 2 * NCHUNK], mybir.dt.float32, tag="sums")

            def compute(xt, w, wv, idx):
                    wa = w - wv
                    # ACT: exact exp on the left wa columns
                    et = epool.tile([P, WAMAX], mybir.dt.bfloat16, tag="et")
                    nc.scalar.activation(
                        out=et[:, :wa],
                        in_=xt[:, :wa],
                        func=mybir.ActivationFunctionType.Exp,
                        accum_out=sums[:, 2 * idx : 2 * idx + 1],
                    )
                    # DVE: Schraudolph exp bits for the right wv columns
                    sc = vpool.tile([P, WVMAX], mybir.dt.int16, tag="sc")
                    nc.vector.tensor_scalar(
                        out=sc[:, :wv],
                        in0=xt[:, wa:w],
                        scalar1=EXP_A,
                        scalar2=EXP_B,
                        op0=mybir.AluOpType.mult,
                        op1=mybir.AluOpType.add,
                    )
                    # two pairwise-add halvings at bf16 2x, then the 1x
                    # accumulating reduce touches only wv/4 elements
                    scf = sc[:, :wv].bitcast(mybir.dt.bfloat16)
                    ht = hpool.tile([P, WVMAX // 2], mybir.dt.bfloat16, tag="ht")
                    nc.vector.tensor_tensor(
                        out=ht[:, : wv // 2],
                        in0=scf[:, : wv // 2],
                        in1=scf[:, wv // 2 :],
                        op=mybir.AluOpType.add,
                    )
                    nc.vector.tensor_tensor(
                        out=ht[:, : wv // 4],
                        in0=ht[:, : wv // 4],
                        in1=ht[:, wv // 4 : wv // 2],
                        op=mybir.AluOpType.add,
                    )
                    nc.vector.tensor_tensor(
                        out=ht[:, : wv // 8],
                        in0=ht[:, : wv // 8],
                        in1=ht[:, wv // 8 : wv // 4],
                        op=mybir.AluOpType.add,
                    )
                    nc.vector.tensor_scalar(
                        out=ht[:, : wv // 8],
                        in0=ht[:, : wv // 8],
                        scalar1=1.0,
                        scalar2=None,
                        op0=mybir.AluOpType.mult,
                        op1=mybir.AluOpType.add,  # accum reduce op
                        accum_out=sums[:, 2 * idx + 1 : 2 * idx + 2],
                    )
                    if idx == NCHUNK - 2:
                        # everything but the last chunk's two accum columns:
                        # overlaps the final chunk's compute
                        nc.sync.dma_start(
                            out=s[:, : 2 * (NCHUNK - 1)],
                            in_=sums[:, : 2 * (NCHUNK - 1)],
                        )

            idx = 0
            for b in range(NBLK):
                col = 0
                if b == 0:
                    # ramp: issue all block-0 DMAs up front, alternating the
                    # two HWDGE rings (sync + scalar queues) so early chunks
                    # land pairwise instead of serializing on one ring
                    tiles = []
                    for j, (w, wv) in enumerate(BLOCK_SPECS[0]):
                        xt = xpool.tile([P, CH], mybir.dt.float8e4, tag="xt")
                        eng = nc.scalar if j % 2 == 1 else nc.sync
                        eng.dma_start(out=xt[:, :w], in_=x[:P, col : col + w])
                        tiles.append((xt, w, wv))
                        col += w
                    for xt, w, wv in tiles:
                        compute(xt, w, wv, idx)
                        idx += 1
                else:
                    for w, wv in BLOCK_SPECS[b]:
                        xt = xpool.tile([P, CH], mybir.dt.float8e4, tag="xt")
                        nc.sync.dma_start(
                            out=xt[:, :w],
                            in_=x[b * P : (b + 1) * P, col : col + w],
                        )
                        compute(xt, w, wv, idx)
                        col += w
                        idx += 1
            nc.scalar.dma_start(
                out=s[:, 2 * (NCHUNK - 1) :], in_=sums[:, 2 * (NCHUNK - 1) :]
            )
    nc.compile()
    return nc


def get_nc():
    if "nc" not in _CACHE:
        _CACHE["nc"] = _build_nc()
    return _CACHE["nc"]


def make_in_maps(predicts: np.ndarray, targets: np.ndarray) -> list[dict]:
    import ml_dtypes

    predicts = np.ascontiguousarray(predicts, dtype=np.float32)
    xq = predicts.astype(ml_dtypes.float8_e4m3)  # RTNE
    return [
        {"x": np.ascontiguousarray(xq[c * R : (c + 1) * R])} for c in range(NCORES)
    ]


def kernel(predicts: np.ndarray, targets: np.ndarray) -> np.ndarray:
    from concourse.bass_utils import run_bass_kernel_spmd

    nc = get_nc()
    predicts = np.ascontiguousarray(predicts, dtype=np.float32)
    targets = np.asarray(targets).astype(np.int64)
    in_maps = make_in_maps(predicts, targets)
    res = run_bass_kernel_spmd(nc, in_maps, list(range(NCORES)))

    # chunk -> block column groups in the [P, 2*NCHUNK] sums output
    bounds = np.cumsum([0] + [len(blk) for blk in BLOCK_SPECS])
    total = np.float64(0.0)
    for c in range(NCORES):
        s = np.asarray(res.results[c]["s"], dtype=np.float64)  # [P, 2*NCHUNK]
        for b in range(NBLK):
            rowsum = s[:, 2 * bounds[b] : 2 * bounds[b + 1]].sum(axis=1)  # [P]
            total += np.log(rowsum).sum()
    picked = predicts[np.arange(BATCH), targets].astype(np.float64)
    return np.asarray((total - picked.sum()) / BATCH, dtype=np.float32)


# revision 19
# speedup vs baseline: 1.1372x; 1.0057x over previous
"""Cross-entropy loss (nn_CrossEntropyLoss) on 8 Trainium2 NeuronCores.

Reference computation (full shapes):
    predicts: [4096, 32000] f32, targets: [4096] int64
    loss = mean_i( log(sum_j exp(predicts[i, j])) - predicts[i, targets[i]] )

Strategy: data-parallel over the batch dim; fp8 input stream; the
sum-of-exp work is column-split between the ACT and DVE engines.

The host rounds predicts to fp8 e4m3 before upload (quartering HBM
traffic to 16 MB/core; the serial input DMA takes ~40us and never
binds).  The exp+row-sum is the real cost: ACT computes exp at a
dtype-independent 1 elem/cycle/lane (153.6 G elem/s), so each core's
16.4M elements would take ~107us on ACT alone.  Each [128, w] chunk is
therefore column-split:
  - ACT: exact exp with accum_out on the left w-wV columns (output to
    a throwaway bf16 scratch; the f32 accumulator is what we keep)
  - DVE: Schraudolph bit-trick exp on the right wV columns --
    tensor_scalar (x*EXP_A + EXP_B) written as int16 produces the
    bf16 bit pattern of ~exp(x) (2x_2p mode, 0.53 cyc/elem); the
    bitcast-bf16 view is then pairwise-added three times (tensor_tensor
    at bf16 2x_1p: 0.28 + 0.14 + 0.07 cyc/elem) and the remaining
    eighth summed by the 1x accumulating tensor_scalar (0.13 cyc/elem)
    -- ~1.17 cyc/elem total, ~105 G elem/s of extra throughput on an
    otherwise idle engine.
The split ratio balances the measured engine rates (ACT 0.833 ns/col +
0.38us/chunk vs DVE 1.17 ns/col + 0.5us/chunk -> ~42% to DVE).  Block 0
ramps chunk widths, with its DMAs issued up-front alternating across
the two HWDGE rings (sync + scalar queues) so the early chunks land
pairwise and both engines start ~10.5us in.  Per-chunk partial row-sums
land in one [128, 2*NCHUNK] f32 tile; all but the last chunk's columns
are DMA'd out under the final chunk's compute, and the last two columns
go out on the scalar queue right after the last accumulator read.

Host side (f64): log of the row-sums (logsumexp), the gather of
predicts[i, targets[i]] from the exact f32 input, and the final mean --
the scalar "all-reduce" across the 8 cores.

Accuracy: fp8 quantizes each logit to ~2^-3.5 relative; the softmax-
weighted average of those i.i.d. perturbations shifts each row's lse by
only ~4e-4 (the 12k-effective-term sum averages them out).  EXP_B is
calibrated so the softmax-weighted mean of approx/true - 1 is zero; the
residual +-3% Schraudolph wiggle averages out the same way.  Measured
end-to-end loss relative error ~1e-4 (tolerance 2e-2).
"""

import sys

import numpy as np

sys.path.insert(0, "/opt/trn_rl_repo")

BATCH = 4096
C = 32000
NCORES = 8
R = BATCH // NCORES  # 512 rows per core
P = 128
NBLK = R // P  # 4 row blocks per core
CH = 16000  # max column chunk (16 KiB/partition in fp8)

# (width, dve_width) per chunk, per block
_S = (CH, 6736)
BLOCK_SPECS = [
    [(3000, 1264), (3000, 1264), (6000, 2528), (6000, 2528), (14000, 5888)],
    [_S, _S],
    [_S, _S],
    [_S, _S],
]
assert all(sum(w for w, _ in blk) == C for blk in BLOCK_SPECS)
assert all(wv % 16 == 0 for blk in BLOCK_SPECS for _, wv in blk)
NCHUNK = sum(len(blk) for blk in BLOCK_SPECS)
WVMAX = max(wv for blk in BLOCK_SPECS for _, wv in blk)
WAMAX = max(w - wv for blk in BLOCK_SPECS for w, wv in blk)

# Schraudolph exp in bf16: exp(x) ~= bitcast_bf16(int16(x * EXP_A + EXP_B)).
# EXP_A = 2^7/ln2 (bf16 exponent layout = f32's, 7-bit mantissa); EXP_B
# calibrated (numpy, softmax-weighted) so the approximation is
# mean-unbiased inside a row sum of exps.
EXP_A = 184.6649652337873
EXP_B = 16249.0

_CACHE: dict = {}


def _build_nc():
    import concourse.bacc as bacc
    import concourse.tile as tile
    from concourse import mybir

    nc = bacc.Bacc(
        "TRN2", target_bir_lowering=False, debug=False, num_devices=NCORES
    )
    x = nc.dram_tensor("x", [R, C], mybir.dt.float8e4, kind="ExternalInput")
    s = nc.dram_tensor(
        "s", [P, 2 * NCHUNK], mybir.dt.float32, kind="ExternalOutput"
    )

    with tile.TileContext(nc) as tc:
        with (
            tc.tile_pool(name="xch", bufs=6) as xpool,
            tc.tile_pool(name="eout", bufs=2) as epool,
            tc.tile_pool(name="bits", bufs=2) as vpool,
            tc.tile_pool(name="half", bufs=2) as hpool,
            tc.tile_pool(name="small", bufs=1) as spool,
        ):
            sums = spool.tile([P, 2 * NCHUNK], mybir.dt.float32, tag="sums")

            def compute(xt, w, wv, idx):
                    wa = w - wv
                    # ACT: exact exp on the left wa columns
                    et = epool.tile([P, WAMAX], mybir.dt.bfloat16, tag="et")
                    nc.scalar.activation(
                        out=et[:, :wa],
                        in_=xt[:, :wa],
                        func=mybir.ActivationFunctionType.Exp,
                        accum_out=sums[:, 2 * idx : 2 * idx + 1],
                    )
                    # DVE: Schraudolph exp bits for the right wv columns
                    sc = vpool.tile([P, WVMAX], mybir.dt.int16, tag="sc")
                    nc.vector.tensor_scalar(
                        out=sc[:, :wv],
                        in0=xt[:, wa:w],
                        scalar1=EXP_A,
                        scalar2=EXP_B,
                        op0=mybir.AluOpType.mult,
                        op1=mybir.AluOpType.add,
                    )
                    # two pairwise-add halvings at bf16 2x, then the 1x
                    # accumulating reduce touches only wv/4 elements
                    scf = sc[:, :wv].bitcast(mybir.dt.bfloat16)
                    ht = hpool.tile([P, WVMAX // 2], mybir.dt.bfloat16, tag="ht")
                    nc.vector.tensor_tensor(
                        out=ht[:, : wv // 2],
                        in0=scf[:, : wv // 2],
                        in1=scf[:, wv // 2 :],
                        op=mybir.AluOpType.add,
                    )
                    nc.vector.tensor_tensor(
                        out=ht[:, : wv // 4],
                        in0=ht[:, : wv // 4],
                        in1=ht[:, wv // 4 : wv // 2],
                        op=mybir.AluOpType.add,
                    )
                    nc.vector.tensor_tensor(
                        out=ht[:, : wv // 8],
                        in0=ht[:, : wv // 8],
                        in1=ht[:, wv // 8 : wv // 4],
                        op=mybir.AluOpType.add,
                    )
                    nc.vector.tensor_scalar(
                        out=ht[:, : wv // 8],
                        in0=ht[:, : wv // 8],
                        scalar1=1.0,
                        scalar2=None,
                        op0=mybir.AluOpType.mult,
                        op1=mybir.AluOpType.add,  # accum reduce op
                        accum_out=sums[:, 2 * idx + 1 : 2 * idx + 2],
                    )
                    if idx == NCHUNK - 2:
                        # everything but the last chunk's two accum columns:
                        # overlaps the final chunk's compute
                        nc.sync.dma_start(
                            out=s[:, : 2 * (NCHUNK - 1)],
                            in_=sums[:, : 2 * (NCHUNK - 1)],
                        )

            idx = 0
            for b in range(NBLK):
                col = 0
                if b == 0:
                    # ramp: issue all block-0 DMAs up front, alternating the
                    # two HWDGE rings (sync + scalar queues) so early chunks
                    # land pairwise instead of serializing on one ring
                    tiles = []
                    for j, (w, wv) in enumerate(BLOCK_SPECS[0]):
                        xt = xpool.tile([P, CH], mybir.dt.float8e4, tag="xt")
                        eng = nc.scalar if j % 2 == 1 else nc.sync
                        eng.dma_start(out=xt[:, :w], in_=x[:P, col : col + w])
                        tiles.append((xt, w, wv))
                        col += w
                    for xt, w, wv in tiles:
                        compute(xt, w, wv, idx)
                        idx += 1
                else:
                    for w, wv in BLOCK_SPECS[b]:
                        xt = xpool.tile([P, CH], mybir.dt.float8e4, tag="xt")
                        nc.sync.dma_start(
                            out=xt[:, :w],
                            in_=x[b * P : (b + 1) * P, col : col + w],
                        )
                        compute(xt, w, wv, idx)
                        col += w
                        idx += 1
            nc.scalar.dma_start(
                out=s[:, 2 * (NCHUNK - 1) :], in_=sums[:, 2 * (NCHUNK - 1) :]
            )
    nc.compile()
    return nc


def get_nc():
    if "nc" not in _CACHE:
        _CACHE["nc"] = _build_nc()
    return _CACHE["nc"]


def make_in_maps(predicts: np.ndarray, targets: np.ndarray) -> list[dict]:
    import ml_dtypes

    predicts = np.ascontiguousarray(predicts, dtype=np.float32)
    xq = predicts.astype(ml_dtypes.float8_e4m3)  # RTNE
    return [
        {"x": np.ascontiguousarray(xq[c * R : (c + 1) * R])} for c in range(NCORES)
    ]


def kernel(predicts: np.ndarray, targets: np.ndarray) -> np.ndarray:
    from concourse.bass_utils import run_bass_kernel_spmd

    nc = get_nc()
    predicts = np.ascontiguousarray(predicts, dtype=np.float32)
    targets = np.asarray(targets).astype(np.int64)
    in_maps = make_in_maps(predicts, targets)
    res = run_bass_kernel_spmd(nc, in_maps, list(range(NCORES)))

    # chunk -> block column groups in the [P, 2*NCHUNK] sums output
    bounds = np.cumsum([0] + [len(blk) for blk in BLOCK_SPECS])
    total = np.float64(0.0)
    for c in range(NCORES):
        s = np.asarray(res.results[c]["s"], dtype=np.float64)  # [P, 2*NCHUNK]
        for b in range(NBLK):
            rowsum = s[:, 2 * bounds[b] : 2 * bounds[b + 1]].sum(axis=1)  # [P]
            total += np.log(rowsum).sum()
    picked = predicts[np.arange(BATCH), targets].astype(np.float64)
    return np.asarray((total - picked.sum()) / BATCH, dtype=np.float32)
